# revision 8
# baseline (speedup 1.0000x reference)
"""LSEP loss kernel for Trainium2 (8 NeuronCores, data-parallel on batch).

loss = log1p( sum_b [ (sum_{c: t=0} e^{x_bc}) * (sum_{c: t=1} e^{-x_bc}) ] ) / B

Encoding: host ships x as bf16 and the target as m16 = t<<15 (uint16, the
bf16 sign-bit position). On device, z = x XOR m16 computes x*(1-2t), so ONE
exp pass yields e^x for negatives and e^{-x} for positives. With
S = sum_c e^z and D = sum_c (e^z XOR m16) = neg_sum - pos_sum, the per-row
product is 4*neg*pos = (S+D)(S-D). Row sums use pair-halving trees of
scalar_tensor_tensor ops (the only DVE instruction with the 4x perf mode;
tensor_reduce always runs 1x); trees go all the way to per-row scalars
before the product (sums of 3-wide partial products would be wrong).
GPSIMD only does the first D-tree stage (its cast ucode measured 14x
slower than DVE in the previous version's trace, but plain bf16 adds are
documented at 0.42 efficiency). Per core: DMA 24.1MB (~73us) should be
the roofline; DVE ~47us, ACT ~42us, GP ~47us.

Output: [128,1] per-core partials of sum 4*s_neg*s_pos; host sums, divides
by 4, applies log1p (the gather/unshard step).
"""

import numpy as np

B = 2_000_000
C = 24
NCORES = 8
P = 128
K = 196
TILES = 10
RPC_RAW = B // NCORES            # 250_000 real rows per core
RPC = P * K * TILES              # 250_880 padded rows per core

_cached = {}


def _build(rows, k, tiles):
    from contextlib import ExitStack

    import concourse.bacc as bacc
    import concourse.tile as tile
    from concourse import mybir

    f32 = mybir.dt.float32
    bf16 = mybir.dt.bfloat16
    u16 = mybir.dt.uint16
    Alu = mybir.AluOpType
    Act = mybir.ActivationFunctionType
    XY = mybir.AxisListType.XY

    nc = bacc.Bacc("TRN2", debug=False, num_devices=NCORES)
    x = nc.dram_tensor("x", [rows, C], bf16, kind="ExternalInput").ap()
    m = nc.dram_tensor("m", [rows, C], u16, kind="ExternalInput").ap()
    out = nc.dram_tensor("o", [P, 1], f32, kind="ExternalOutput").ap()

    xv = x.rearrange("(i p k) c -> i p k c", p=P, k=k)
    mv = m.rearrange("(i p k) c -> i p k c", p=P, k=k)

    def tt(eng, out_, in0, in1, op):
        return eng.tensor_tensor(out=out_, in0=in0, in1=in1, op=op)

    with tile.TileContext(nc) as tc, ExitStack() as ctx:
        io = ctx.enter_context(tc.tile_pool(name="io", bufs=3))
        ep = ctx.enter_context(tc.tile_pool(name="ep", bufs=3))
        tp = ctx.enter_context(tc.tile_pool(name="tp", bufs=2))
        accp = ctx.enter_context(tc.tile_pool(name="accp", bufs=1))
        acc = accp.tile([P, tiles, k], bf16)  # 4*neg*pos per row, all tiles
        V = nc.vector
        G = nc.gpsimd
        for i in range(tiles):
            xt = io.tile([P, k, C], bf16, tag="x")
            mt = io.tile([P, k, C], u16, tag="m")
            nc.sync.dma_start(out=xt, in_=xv[i])
            nc.sync.dma_start(out=mt, in_=mv[i])
            xu = xt.bitcast(u16)
            tt(V, xu, xu, mt, Alu.bitwise_xor)               # z = x^(t<<15)
            e = ep.tile([P, k, C], bf16, tag="e")
            nc.scalar.activation(out=e, in_=xt, func=Act.Exp)
            se = ep.tile([P, k, C], bf16, tag="se")
            tt(V, se.bitcast(u16), e.bitcast(u16), mt, Alu.bitwise_xor)
            # trees to per-row scalars: 24 -> 12 -> 6 -> 2 -> 1
            s1 = tp.tile([P, k, 12], bf16, tag="s1")
            d1 = tp.tile([P, k, 12], bf16, tag="d1")
            tt(V, s1, e[:, :, 0:12], e[:, :, 12:24], Alu.add)
            tt(G, d1, se[:, :, 0:12], se[:, :, 12:24], Alu.add)
            s2 = tp.tile([P, k, 6], bf16, tag="s2")
            d2 = tp.tile([P, k, 6], bf16, tag="d2")
            tt(V, s2, s1[:, :, 0:6], s1[:, :, 6:12], Alu.add)
            tt(G, d2, d1[:, :, 0:6], d1[:, :, 6:12], Alu.add)
            s3 = tp.tile([P, k, 2], bf16, tag="s3")
            d3 = tp.tile([P, k, 2], bf16, tag="d3")
            tt(V, s3, s2[:, :, 0:2], s2[:, :, 2:4], Alu.add)
            tt(G, d3, d2[:, :, 0:2], d2[:, :, 2:4], Alu.add)
            tt(V, s3, s3, s2[:, :, 4:6], Alu.add)
            tt(G, d3, d3, d2[:, :, 4:6], Alu.add)
            sS = tp.tile([P, k], bf16, tag="sS")
            dS = tp.tile([P, k], bf16, tag="dS")
            tt(V, sS, s3[:, :, 0], s3[:, :, 1], Alu.add)
            tt(V, dS, d3[:, :, 0], d3[:, :, 1], Alu.add)
            u = tp.tile([P, k], bf16, tag="u")
            v = tp.tile([P, k], bf16, tag="v")
            tt(V, u, sS, dS, Alu.add)                        # 2*neg
            tt(V, v, sS, dS, Alu.subtract)                   # 2*pos
            tt(V, acc[:, i], u, v, Alu.mult)                 # 4*neg*pos
        a1 = accp.tile([P, 1], f32)
        nc.vector.tensor_reduce(out=a1, in_=acc, axis=XY, op=Alu.add)
        nc.sync.dma_start(out=out, in_=a1)
    nc.compile()
    return nc


def _get_nc():
    key = (RPC, K, TILES)
    if key not in _cached:
        _cached[key] = _build(RPC, K, TILES)
    return _cached[key]


def _f32_to_bf16_u16(a):
    # round-to-nearest-even f32 -> bf16, as uint16 bit pattern
    u = a.view(np.uint32)
    r = ((u >> 16) & 1) + np.uint32(0x7FFF)
    return ((u + r) >> 16).astype(np.uint16)


def _shard(input, target):
    import ml_dtypes

    xb = _f32_to_bf16_u16(input).view(ml_dtypes.bfloat16)
    mb = (target << 15).astype(np.uint16)
    in_maps = []
    for c in range(NCORES):
        xs = np.zeros((RPC, C), ml_dtypes.bfloat16)
        ms = np.zeros((RPC, C), np.uint16)
        xs[:RPC_RAW] = xb[c * RPC_RAW : (c + 1) * RPC_RAW]
        ms[:RPC_RAW] = mb[c * RPC_RAW : (c + 1) * RPC_RAW]
        in_maps.append({"x": xs, "m": ms})
    return in_maps


_last_results = None


def kernel(input, target):
    global _last_results
    input = np.ascontiguousarray(np.asarray(input, dtype=np.float32))
    target = np.ascontiguousarray(np.asarray(target, dtype=np.int32))
    assert input.shape == (B, C) and target.shape == (B, C)

    from concourse.bass_utils import run_bass_kernel_spmd

    nc = _get_nc()
    in_maps = _shard(input, target)
    res = run_bass_kernel_spmd(nc, in_maps, core_ids=list(range(NCORES)))
    _last_results = res
    total = float(np.sum([r["o"] for r in res.results], dtype=np.float64)) / 4.0
    return np.asarray(np.log1p(total) / B, dtype=np.float32)


# revision 9
# speedup vs baseline: 1.0704x; 1.0704x over previous
"""LSEP loss kernel for Trainium2 (8 NeuronCores, data-parallel on batch).

loss = log1p( sum_b [ (sum_{c: t=0} e^{x_bc}) * (sum_{c: t=1} e^{-x_bc}) ] ) / B

Encoding: host ships x as bf16 and the target as m16 = t<<15 (uint16, the
bf16 sign-bit position). On device, z = x XOR m16 computes x*(1-2t), so ONE
exp pass yields e^x for negatives and e^{-x} for positives. With
S = sum_c e^z and D = sum_c (e^z XOR m16) = neg_sum - pos_sum, the per-row
product is 4*neg*pos = (S+D)(S-D).

Row sums: pair-halving tensor_tensor adds (the 2x DVE mode; tensor_reduce
and scalar_tensor_tensor both measured 1x on HW) down to 6 wide, then one
1x tensor_reduce to f32 row scalars. GPSIMD (measured 2.2ns/elem for bf16
adds) takes the D-tree's first two stages. Engines execute their queues
in order, so the loop is software-pipelined by hand: each tile's
GP-dependent tail (dred/u/v/prod) and ACT-dependent ops (se/s-tree) are
emitted one tile later than their producers to keep every queue busy.

Output: [128,1] per-core partials of sum 4*s_neg*s_pos; host sums, divides
by 4, applies log1p (the gather/unshard step).
"""

import numpy as np

B = 2_000_000
C = 24
NCORES = 8
P = 128
K = 196
TILES = 10
RPC_RAW = B // NCORES            # 250_000 real rows per core
RPC = P * K * TILES              # 250_880 padded rows per core

_cached = {}


def _build(rows, k, tiles):
    from contextlib import ExitStack

    import concourse.bacc as bacc
    import concourse.tile as tile
    from concourse import mybir

    f32 = mybir.dt.float32
    bf16 = mybir.dt.bfloat16
    u16 = mybir.dt.uint16
    Alu = mybir.AluOpType
    Act = mybir.ActivationFunctionType
    X = mybir.AxisListType.X
    XY = mybir.AxisListType.XY

    nc = bacc.Bacc("TRN2", debug=False, num_devices=NCORES)
    x = nc.dram_tensor("x", [rows, C], bf16, kind="ExternalInput").ap()
    m = nc.dram_tensor("m", [rows, C], u16, kind="ExternalInput").ap()
    out = nc.dram_tensor("o", [P, 1], f32, kind="ExternalOutput").ap()

    xv = x.rearrange("(i p k) c -> i p k c", p=P, k=k)
    mv = m.rearrange("(i p k) c -> i p k c", p=P, k=k)

    with tile.TileContext(nc) as tc, ExitStack() as ctx:
        io = ctx.enter_context(tc.tile_pool(name="io", bufs=3))
        ep = ctx.enter_context(tc.tile_pool(name="ep", bufs=3))
        tp = ctx.enter_context(tc.tile_pool(name="tp", bufs=3))
        accp = ctx.enter_context(tc.tile_pool(name="accp", bufs=1))
        acc = accp.tile([P, tiles, k], f32)  # 4*neg*pos per row, all tiles
        V = nc.vector
        G = nc.gpsimd

        st = {}  # per-tile tiles carried across pipeline stages

        def stage_a(i):
            # DMA in, z = x ^ m, exp on ACT
            xt = io.tile([P, k, C], bf16, tag="x")
            mt = io.tile([P, k, C], u16, tag="m")
            nc.sync.dma_start(out=xt, in_=xv[i])
            nc.sync.dma_start(out=mt, in_=mv[i])
            xu = xt.bitcast(u16)
            V.tensor_tensor(out=xu, in0=xu, in1=mt, op=Alu.bitwise_xor)
            e = ep.tile([P, k, C], bf16, tag="e")
            nc.scalar.activation(out=e, in_=xt, func=Act.Exp)
            st[i] = {"e": e, "m": mt}

        def stage_b(i):
            # se = e ^ m; S-tree on DVE, D-tree head on GPSIMD
            e, mt = st[i]["e"], st[i]["m"]
            se = ep.tile([P, k, C], bf16, tag="se")
            V.tensor_tensor(out=se.bitcast(u16), in0=e.bitcast(u16), in1=mt,
                            op=Alu.bitwise_xor)
            d1 = tp.tile([P, k, 12], bf16, tag="d1")
            G.tensor_add(d1, se[:, :, 0:12], se[:, :, 12:24])
            d2 = tp.tile([P, k, 6], bf16, tag="d2")
            G.tensor_add(d2, d1[:, :, 0:6], d1[:, :, 6:12])
            s1 = tp.tile([P, k, 12], bf16, tag="s1")
            V.tensor_add(s1, e[:, :, 0:12], e[:, :, 12:24])
            s2 = tp.tile([P, k, 6], bf16, tag="s2")
            V.tensor_add(s2, s1[:, :, 0:6], s1[:, :, 6:12])
            sS = tp.tile([P, k], f32, tag="sS")
            V.tensor_reduce(out=sS, in_=s2, axis=X, op=Alu.add)
            st[i].update(d2=d2, sS=sS)

        def stage_c(i):
            # D row scalars (reads GPSIMD's d2), u/v/product
            d2, sS = st[i]["d2"], st[i]["sS"]
            dS = tp.tile([P, k], f32, tag="dS")
            V.tensor_reduce(out=dS, in_=d2, axis=X, op=Alu.add)
            u = tp.tile([P, k], f32, tag="u")
            v = tp.tile([P, k], f32, tag="v")
            V.tensor_add(u, sS, dS)                          # 2*neg
            V.tensor_sub(v, sS, dS)                          # 2*pos
            V.tensor_tensor(out=acc[:, i], in0=u, in1=v, op=Alu.mult)
            del st[i]

        stage_a(0)
        stage_a(1)
        stage_b(0)
        for i in range(tiles):
            if i + 2 < tiles:
                stage_a(i + 2)
            if i + 1 < tiles:
                stage_b(i + 1)
            stage_c(i)
        a1 = accp.tile([P, 1], f32)
        nc.vector.tensor_reduce(out=a1, in_=acc, axis=XY, op=Alu.add)
        nc.sync.dma_start(out=out, in_=a1)
    nc.compile()
    return nc


def _get_nc():
    key = (RPC, K, TILES)
    if key not in _cached:
        _cached[key] = _build(RPC, K, TILES)
    return _cached[key]


def _f32_to_bf16_u16(a):
    # round-to-nearest-even f32 -> bf16, as uint16 bit pattern
    u = a.view(np.uint32)
    r = ((u >> 16) & 1) + np.uint32(0x7FFF)
    return ((u + r) >> 16).astype(np.uint16)


def _shard(input, target):
    import ml_dtypes

    xb = _f32_to_bf16_u16(input).view(ml_dtypes.bfloat16)
    mb = (target << 15).astype(np.uint16)
    in_maps = []
    for c in range(NCORES):
        xs = np.zeros((RPC, C), ml_dtypes.bfloat16)
        ms = np.zeros((RPC, C), np.uint16)
        xs[:RPC_RAW] = xb[c * RPC_RAW : (c + 1) * RPC_RAW]
        ms[:RPC_RAW] = mb[c * RPC_RAW : (c + 1) * RPC_RAW]
        in_maps.append({"x": xs, "m": ms})
    return in_maps


_last_results = None


def kernel(input, target):
    global _last_results
    input = np.ascontiguousarray(np.asarray(input, dtype=np.float32))
    target = np.ascontiguousarray(np.asarray(target, dtype=np.int32))
    assert input.shape == (B, C) and target.shape == (B, C)

    from concourse.bass_utils import run_bass_kernel_spmd

    nc = _get_nc()
    in_maps = _shard(input, target)
    res = run_bass_kernel_spmd(nc, in_maps, core_ids=list(range(NCORES)))
    _last_results = res
    total = float(np.sum([r["o"] for r in res.results], dtype=np.float64)) / 4.0
    return np.asarray(np.log1p(total) / B, dtype=np.float32)


# revision 12
# speedup vs baseline: 1.1820x; 1.1042x over previous
"""LSEP loss kernel for Trainium2 (8 NeuronCores, data-parallel on batch).

loss = log1p( sum_b [ (sum_{c: t=0} e^{x_bc}) * (sum_{c: t=1} e^{-x_bc}) ] ) / B

Encoding: host ships x as bf16 and the target as m16 = t<<15 (uint16, the
bf16 sign-bit position). On device, z = x XOR m16 computes x*(1-2t), so ONE
exp pass yields e^x for negatives and e^{-x} for positives. With
S = sum_c e^z and D = sum_c (e^z XOR m16) = neg_sum - pos_sum, the per-row
product is 4*neg*pos = (S+D)(S-D).

Row sums: pair-halving tensor_tensor adds (the 2x DVE mode; tensor_reduce
and scalar_tensor_tensor both measured 1x on HW) down to 6 wide, then one
1x tensor_reduce to f32 row scalars. GPSIMD (measured 2.2ns/elem for bf16
adds) takes the D-tree's first two stages. Engines execute their queues
in order, so the loop is software-pipelined by hand: each tile's
GP-dependent tail (dred/u/v/prod) and ACT-dependent ops (se/s-tree) are
emitted one tile later than their producers to keep every queue busy.

Output: [128,1] per-core partials of sum 4*s_neg*s_pos; host sums, divides
by 4, applies log1p (the gather/unshard step).
"""

import numpy as np

B = 2_000_000
C = 24
NCORES = 8
P = 128
K = 196
TILES = 10
RPC_RAW = B // NCORES            # 250_000 real rows per core
RPC = P * K * TILES              # 250_880 padded rows per core

_cached = {}


def _build(rows, k, tiles):
    from contextlib import ExitStack

    import concourse.bacc as bacc
    import concourse.tile as tile
    from concourse import mybir

    f32 = mybir.dt.float32
    bf16 = mybir.dt.bfloat16
    u16 = mybir.dt.uint16
    Alu = mybir.AluOpType
    Act = mybir.ActivationFunctionType
    X = mybir.AxisListType.X
    XY = mybir.AxisListType.XY

    nc = bacc.Bacc("TRN2", debug=False, num_devices=NCORES)
    x = nc.dram_tensor("x", [rows, C], bf16, kind="ExternalInput").ap()
    m = nc.dram_tensor("m", [rows, C], u16, kind="ExternalInput").ap()
    out = nc.dram_tensor("o", [P, 2], f32, kind="ExternalOutput").ap()

    xv = x.rearrange("(i p k) c -> i p k c", p=P, k=k)
    mv = m.rearrange("(i p k) c -> i p k c", p=P, k=k)

    with tile.TileContext(nc) as tc, ExitStack() as ctx:
        io = ctx.enter_context(tc.tile_pool(name="io", bufs=3))
        ep = ctx.enter_context(tc.tile_pool(name="ep", bufs=3))
        tp = ctx.enter_context(tc.tile_pool(name="tp", bufs=3))
        accp = ctx.enter_context(tc.tile_pool(name="accp", bufs=1))
        acc1 = accp.tile([P, tiles, k], f32)  # (2*neg)^2 per row, all tiles
        acc2 = accp.tile([P, tiles, k], f32)  # (2*pos... actually D^2)
        V = nc.vector
        G = nc.gpsimd

        st = {}  # per-tile tiles carried across pipeline stages

        def stage_a(i):
            # DMA in, z = x ^ m, exp on ACT
            xt = io.tile([P, k, C], bf16, tag="x")
            mt = io.tile([P, k, C], u16, tag="m")
            nc.sync.dma_start(out=xt, in_=xv[i])
            nc.sync.dma_start(out=mt, in_=mv[i])
            xu = xt.bitcast(u16)
            V.tensor_tensor(out=xu, in0=xu, in1=mt, op=Alu.bitwise_xor)
            e = ep.tile([P, k, C], bf16, tag="e")
            nc.scalar.activation(out=e, in_=xt, func=Act.Exp)
            st[i] = {"e": e, "m": mt}

        def stage_b(i):
            # se = e ^ m; S-tree fully on DVE, D-tree head on GPSIMD
            e, mt = st[i]["e"], st[i]["m"]
            se = ep.tile([P, k, C], bf16, tag="se")
            V.tensor_tensor(out=se.bitcast(u16), in0=e.bitcast(u16), in1=mt,
                            op=Alu.bitwise_xor)
            d1 = tp.tile([P, k, 12], bf16, tag="d1")
            G.tensor_add(d1, se[:, :, 0:12], se[:, :, 12:24])
            s1 = tp.tile([P, k, 12], bf16, tag="s1")
            V.tensor_add(s1, e[:, :, 0:12], e[:, :, 12:24])
            s2 = tp.tile([P, k, 6], bf16, tag="s2")
            V.tensor_add(s2, s1[:, :, 0:6], s1[:, :, 6:12])
            sS = tp.tile([P, k], f32, tag="sS")
            V.tensor_reduce(out=sS, in_=s2, axis=X, op=Alu.add)
            V.tensor_tensor(out=acc1[:, i], in0=sS, in1=sS, op=Alu.mult)
            st[i].update(d1=d1)

        def stage_c(i):
            # D tail (reads GPSIMD's d1 from a full tile ago)
            d1 = st[i]["d1"]
            d2 = tp.tile([P, k, 6], bf16, tag="d2")
            V.tensor_add(d2, d1[:, :, 0:6], d1[:, :, 6:12])
            dS = tp.tile([P, k], f32, tag="dS")
            V.tensor_reduce(out=dS, in_=d2, axis=X, op=Alu.add)
            V.tensor_tensor(out=acc2[:, i], in0=dS, in1=dS, op=Alu.mult)
            del st[i]

        stage_a(0)
        stage_a(1)
        stage_b(0)
        for i in range(tiles):
            if i + 2 < tiles:
                stage_a(i + 2)
            if i + 1 < tiles:
                stage_b(i + 1)
            stage_c(i)
        a1 = accp.tile([P, 2], f32)
        nc.vector.tensor_reduce(out=a1[:, 0:1], in_=acc1, axis=XY, op=Alu.add)
        nc.vector.tensor_reduce(out=a1[:, 1:2], in_=acc2, axis=XY, op=Alu.add)
        nc.sync.dma_start(out=out, in_=a1)
    nc.compile()
    return nc


def _get_nc():
    key = (RPC, K, TILES)
    if key not in _cached:
        _cached[key] = _build(RPC, K, TILES)
    return _cached[key]


def _f32_to_bf16_u16(a):
    # round-to-nearest-even f32 -> bf16, as uint16 bit pattern
    u = a.view(np.uint32)
    r = ((u >> 16) & 1) + np.uint32(0x7FFF)
    return ((u + r) >> 16).astype(np.uint16)


def _shard(input, target):
    import ml_dtypes

    xb = _f32_to_bf16_u16(input).view(ml_dtypes.bfloat16)
    mb = (target << 15).astype(np.uint16)
    in_maps = []
    for c in range(NCORES):
        xs = np.zeros((RPC, C), ml_dtypes.bfloat16)
        ms = np.zeros((RPC, C), np.uint16)
        xs[:RPC_RAW] = xb[c * RPC_RAW : (c + 1) * RPC_RAW]
        ms[:RPC_RAW] = mb[c * RPC_RAW : (c + 1) * RPC_RAW]
        in_maps.append({"x": xs, "m": ms})
    return in_maps


_last_results = None


def kernel(input, target):
    global _last_results
    input = np.ascontiguousarray(np.asarray(input, dtype=np.float32))
    target = np.ascontiguousarray(np.asarray(target, dtype=np.int32))
    assert input.shape == (B, C) and target.shape == (B, C)

    from concourse.bass_utils import run_bass_kernel_spmd

    nc = _get_nc()
    in_maps = _shard(input, target)
    res = run_bass_kernel_spmd(nc, in_maps, core_ids=list(range(NCORES)))
    _last_results = res
    ssum = float(np.sum([r["o"][:, 0] for r in res.results], dtype=np.float64))
    dsum = float(np.sum([r["o"][:, 1] for r in res.results], dtype=np.float64))
    total = (ssum - dsum) / 4.0
    return np.asarray(np.log1p(total) / B, dtype=np.float32)
